# revision 15
# baseline (speedup 1.0000x reference)
"""Trainium2 Bass kernel for MultiHeadHypergraphAttention.

Problem: queries (4, 1024, 512), keys (4, 4096, 512), incidence (4, 1024, 4096) i32,
torch-Linear Q/K/V/O projections, per-head masked softmax attention.

Sharding (8 cores): batch (4) x head-group (2 groups of 4 heads).
Core c handles batch b = c//2, head group g = c%2 and produces the partial
output projection for its 4 heads; the host sums the two partials per batch.

Device-side layout ("scores transposed"): S^T is computed with nodes on
partitions and edges on the free axis, so the incidence mask (host-marshalled
to (nodes, edges) bf16) is applied in its natural layout as a DVE multiply
of exp(s/8), and attention weights P^T feed the attn@V matmul directly as
the moving operand (V' stationary), producing O^T with head dims on
partitions - exactly the orientation the output projection needs.

Softmax normalization is folded into the output: V is augmented with a
ones-column so attn@V also produces row sums; O^T rows are divided by those
sums via a fast approximate reciprocal and a single-DRAM-bounce partition
broadcast. Masked entries are exact zeros (bf16 mask multiply).

All inputs arrive pre-cast from the host (bf16 activations/weights/mask),
so no on-chip dtype conversion of inputs is needed and HBM traffic drops
from ~30 MB (f32/i32) to ~13.5 MB per core. All matmuls run bf16 with f32
PSUM accumulation; the per-head scores matmul contracts over the full 128
partitions via zero-padded head-pair Q^T tiles. The engine balance is
ACT(exp)-bound: 128 exps of [128, 1024] ~ 141 us busy; PE ~ 119 us;
DVE ~ 120 us; DMA ~ 41 us. Heads run sequentially; each head's softmax
normalization hides inside the next head's stream.
"""

import sys
import os

for _p in ("/opt/trn_rl_repo",):
    if _p not in sys.path and os.path.isdir(_p):
        sys.path.insert(0, _p)

import numpy as np
import ml_dtypes
from contextlib import ExitStack

import concourse.bass as bass
import concourse.mybir as mybir
import concourse.tile as tile
from concourse import bacc
from concourse.bass_utils import run_bass_kernel_spmd

BF16 = mybir.dt.bfloat16
F32 = mybir.dt.float32

BF16NP = np.dtype(ml_dtypes.bfloat16)

BS, E, N, D = 4, 1024, 4096, 512
HL = 4                   # heads per core (local)
NCHUNK = N // 128        # 32 node chunks
ECHUNK = E // 128        # 8

LAST_EXEC_TIME_NS = None
_CACHED_NC = None


def _build_nc():
    nc = bacc.Bacc("TRN2", target_bir_lowering=False, debug=False, num_devices=8)

    qT_d = nc.dram_tensor("qT", (128, 4096), BF16, kind="ExternalInput").ap()
    kTw_d = nc.dram_tensor("kTw", (8, 128, 2048), BF16, kind="ExternalInput").ap()
    mB_d = nc.dram_tensor("mB", (NCHUNK // 2, 128, 2 * E), BF16, kind="ExternalInput").ap()
    # all projection weights packed [128, 4096] so the weight load is one
    # fat-line transfer: cols = wq(4x256) | wk(4x256) | wv(4x256) | wo(2x512)
    wAll_d = nc.dram_tensor("wAll", (128, 4096), BF16, kind="ExternalInput").ap()
    bias_d = nc.dram_tensor("bias4", (128, 4), F32, kind="ExternalInput").ap()
    out_d = nc.dram_tensor("out", (E, 512), F32, kind="ExternalOutput").ap()

    with tile.TileContext(nc) as tc, ExitStack() as ctx:
        persist = ctx.enter_context(tc.tile_pool(name="persist", bufs=1))
        work = ctx.enter_context(tc.tile_pool(name="work", bufs=1))
        ps = ctx.enter_context(tc.tile_pool(name="ps", bufs=1, space="PSUM"))
        dpool = ctx.enter_context(tc.tile_pool(name="dpool", bufs=1, space="DRAM"))

        # ---------------- constants ----------------
        QTs = [persist.tile([128, E], BF16, tag=f"QTs{l}", name=f"QTs{l}")
               for l in range(HL)]
        for l in range(HL):
            r = l % 2
            zsl = slice(64 * (1 - r), 64 * (1 - r) + 64)
            nc.vector.memset(QTs[l][zsl, :], 0.0)
        # V' bf16: [128 nodes, chunk, head, 65] ; col 64 = ones (row sums)
        Vs = persist.tile([128, NCHUNK * HL * 65], BF16, tag="Vs")
        Vs4 = Vs.rearrange("p (n h c) -> p n h c", n=NCHUNK, h=HL)
        nc.vector.memset(Vs4[:, :, :, 64:65], 1.0)

        # ------------- weight loads: one fat transfer each ----------------
        wAll = persist.tile([128, 4096], BF16, tag="wAll")
        nc.sync.dma_start(out=wAll, in_=wAll_d)
        biasT = persist.tile([128, 4], F32, tag="bias4")
        nc.gpsimd.dma_start(out=biasT, in_=bias_d)
        wqTb = [wAll[:, c * 256:(c + 1) * 256] for c in range(4)]
        wkTb = [wAll[:, 1024 + c * 256:1024 + (c + 1) * 256] for c in range(4)]
        wvTb = [wAll[:, 2048 + c * 256:2048 + (c + 1) * 256] for c in range(4)]
        woTb = [wAll[:, 3072 + p * 512:3072 + (p + 1) * 512] for p in range(2)]
        bqs = [biasT[:, p:p + 1] for p in range(2)]
        bks = [biasT[:, 2 + p:3 + p] for p in range(2)]

        # ------------- streaming input loads ------------------------------
        # mask chunks stream on the SWDGE queue (issued first so chunk 0
        # lands as early as possible); the HWDGE queue carries window 0,
        # then qT, then windows 1-7. Each kT window is one 512KB transfer
        # with 4KB contiguous lines (the four 128-row D-blocks side by
        # side in the free axis) so the DGE uses fat packets.
        Mb = persist.tile([128, NCHUNK * E], BF16, tag="Mb")
        kWins = [persist.tile([128, 2048], BF16, tag=f"kW{w}", name=f"kW{w}")
                 for w in range(8)]
        qAll = persist.tile([128, 4096], BF16, tag="qAll")
        qTb = [qAll[:, c * E:(c + 1) * E] for c in range(4)]
        nc.gpsimd.dma_start(out=qAll[:, 2 * E:], in_=qT_d[:, 2 * E:])
        for n in range(NCHUNK // 2):
            nc.gpsimd.dma_start(out=Mb[:, n * 2 * E:(n + 1) * 2 * E],
                                in_=mB_d[n])
        nc.sync.dma_start(out=qAll[:, 0:2 * E], in_=qT_d[:, 0:2 * E])
        for w in range(8):
            nc.sync.dma_start(out=kWins[w], in_=kTw_d[w])

        # ---------------- Q projection ----------------
        # QTs[l] (128, 1024) bf16: rows [64r, 64r+64) = head l's Q^T, rest 0
        # (l = 2p + r), so scores matmuls contract over the full 128
        # partitions against KTs[p]. Pair 1's Q/K projections are deferred
        # into the ACT-bound head-1/2 streams (only heads 2,3 need them).
        def proj_q(p):
            qp = ps.tile([128, E], F32, tag="st", bufs=2, name=f"qp{p}")
            for c in range(4):
                for e2 in range(2):
                    nc.tensor.matmul(
                        qp[:, e2 * 512:(e2 + 1) * 512],
                        wqTb[c][:, p * 128:(p + 1) * 128],
                        qTb[c][:, e2 * 512:(e2 + 1) * 512],
                        start=(c == 0), stop=(c == 3))
            for r in range(2):
                sl = slice(64 * r, 64 * r + 64)
                nc.vector.tensor_scalar_add(QTs[2 * p + r][sl, :], qp[sl, :],
                                            bqs[p][sl, :])

        proj_q(0)

        # ------------- K/V projections -----------------------------------
        KTs = [persist.tile([128, N], BF16, tag=f"KTs{p}", name=f"KTs{p}")
               for p in range(2)]
        pairN = [persist.tile([128, E], BF16, tag=f"pairN{p}", name=f"pairN{p}")
                 for p in range(2)]

        def proj_k(w, p):
            kp = ps.tile([128, 512], F32, tag="st", bufs=2, name=f"kp{p}_{w}")
            for c in range(4):
                nc.tensor.matmul(
                    kp, wkTb[c][:, p * 128:(p + 1) * 128],
                    kWins[w][:, c * 512:(c + 1) * 512],
                    start=(c == 0), stop=(c == 3))
            nc.vector.tensor_scalar_add(
                KTs[p][:, w * 512:(w + 1) * 512], kp, bks[p])

        def proj_v(n):
            w, j = divmod(n, 4)
            vp = ps.tile([128, 256], F32, tag="st", bufs=2, name=f"vp{n}")
            for c in range(4):
                blk = kWins[w][:, c * 512 + j * 128:c * 512 + j * 128 + 128]
                nc.tensor.matmul(vp, blk,
                                 wvTb[c], start=(c == 0), stop=(c == 3))
            dst = Vs4[:, n, :, 0:64]
            src = vp.rearrange("p (h c) -> p h c", h=4)
            nc.vector.tensor_copy(dst, src)

        # ------------- attention helpers ---------------------------------
        oTs = {}
        Ps = {}

        def score_part(l, n):
            # scores + exp + mask for (head l, node chunk n) -> P^T in Ps
            p = l // 2
            st = ps.tile([128, E], F32, tag="st", bufs=2, name=f"st{l}_{n}")
            kblk = KTs[p][:, n * 128:(n + 1) * 128]
            for e2 in range(2):
                sl = slice(e2 * 512, (e2 + 1) * 512)
                nc.tensor.matmul(st[:, sl], kblk, QTs[l][:, sl],
                                 start=True, stop=True)
            Praw = work.tile([128, E], BF16, tag="Praw", bufs=6,
                             name=f"Praw{l}_{n}")
            nc.scalar.activation(Praw, st, mybir.ActivationFunctionType.Exp,
                                 bias=0.0, scale=0.125)
            P = work.tile([128, E], BF16, tag="P", bufs=6, name=f"P{l}_{n}")
            nc.vector.tensor_mul(P, Praw, Mb[:, n * E:(n + 1) * E])
            Ps[(l, n)] = P

        def av_part(l, n):
            # attn @ V' for (head l, node chunk n), accumulating into oTs[l]
            P = Ps.pop((l, n))
            vblk = Vs4[:, n, l]
            for e2 in range(2):
                sl = slice(e2 * 512, (e2 + 1) * 512)
                nc.tensor.matmul(oTs[l][:, sl], vblk, P[:, sl],
                                 start=(n == 0), stop=(n == NCHUNK - 1))

        def head_seq(l):
            seq = []
            for n in range(NCHUNK):
                seq.append(lambda l=l, n=n: score_part(l, n))
                if n > 0:
                    seq.append(lambda l=l, n=n - 1: av_part(l, n))
            seq.append(lambda l=l: av_part(l, NCHUNK - 1))
            return seq

        # ------------- normalization (via DRAM bounces, as baseline) -----
        norm_state = {}

        def norm_stage1(l):
            # copy the exp-sum row out of PSUM, bounce to DRAM and back
            # reshaped (64, 16) so the reciprocal runs 64 lanes wide
            sums = work.tile([1, E], F32, tag="sums", bufs=2, name=f"sums{l}")
            nc.vector.tensor_copy(sums, oTs[l][64:65, :])
            sums_d = dpool.tile([1, E], F32, tag="sums_d", bufs=2,
                                name=f"sums_d{l}")
            nc.sync.dma_start(out=sums_d, in_=sums)
            sums64 = work.tile([64, 16], F32, tag="sums64", bufs=2,
                               name=f"sums64{l}")
            nc.sync.dma_start(
                out=sums64, in_=sums_d.rearrange("one (p k) -> (one p) k", p=64))
            norm_state[l] = sums64

        def norm_stage2(l):
            sums64 = norm_state.pop(l)
            recip64 = work.tile([64, 16], F32, tag="recip64", bufs=2,
                                name=f"recip64{l}")
            nc.vector.reciprocal(recip64, sums64)
            rec_d = dpool.tile([64, 16], F32, tag="rec_d", bufs=2,
                               name=f"rec_d{l}")
            nc.sync.dma_start(out=rec_d, in_=recip64)
            norm_state[l] = rec_d

        def norm_stage3(l):
            rec_row = norm_state[l].rearrange("p k -> (p k)").unsqueeze(0)
            recb = work.tile([64, E], F32, tag="recb", bufs=2, name=f"recb{l}")
            nc.sync.dma_start(out=recb, in_=rec_row.to_broadcast((64, E)))
            norm_state[l] = recb

        def norm_stage4(l):
            p, r = l // 2, l % 2
            recb = norm_state.pop(l)
            nc.vector.tensor_mul(pairN[p][64 * r:64 * r + 64, :],
                                 oTs[l][0:64, :], recb)

        NORM_STAGES = (norm_stage1, norm_stage2, norm_stage3, norm_stage4)

        def normalize(l):
            for s in NORM_STAGES:
                s(l)

        # ------------- merged pipeline -----------------------------------
        for l in (0, 1):
            oTs[l] = ps.tile([65, E], F32, tag="outT", bufs=2, name=f"oT{l}")

        # interleaved heads 0+1 trail the K/V projection windows by one
        # window: both oT accumulators are live, so both heads' chunks pump
        # the ACT engine through the PE-bound projection phase
        proj_q(1)
        h01 = []
        for n in range(NCHUNK):
            h01.append(lambda n=n: score_part(0, n))
            h01.append(lambda n=n: score_part(1, n))
            if n > 0:
                h01.append(lambda n=n - 1: av_part(0, n))
                h01.append(lambda n=n - 1: av_part(1, n))
        h01.append(lambda: av_part(0, NCHUNK - 1))
        h01.append(lambda: av_part(1, NCHUNK - 1))
        hi = 0
        for w in range(8):
            steps = [lambda w=w: proj_k(w, 0), lambda w=w: proj_k(w, 1)] + \
                    [lambda n=n: proj_v(n) for n in range(4 * w, 4 * w + 4)]
            for i, step in enumerate(steps):
                if w > 0 and hi < len(h01):
                    for _ in range(2):
                        if hi < len(h01):
                            h01[hi]()
                            hi += 1
                step()
        while hi < len(h01):
            h01[hi]()
            hi += 1

        # head 0's normalization runs at the phase boundary (its oT slot is
        # needed by head 2's accumulator); head 1's and head 2's norms hide
        # inside the following streams
        normalize(0)
        for l in (2, 3):
            oTs[l] = ps.tile([65, E], F32, tag="outT", bufs=2,
                             name=f"oT{l}")
            stages = {2: 0, 8: 1, 14: 2, 20: 3}
            for idx, item in enumerate(head_seq(l)):
                item()
                if idx in stages:
                    NORM_STAGES[stages[idx]](l - 1)
        normalize(3)

        # ---------------- output projection (partial) --------------------
        # fin tiles rotate over both PSUM tags (4 slots) and the result is
        # DMAed straight from PSUM, alternating queues
        for e in range(ECHUNK):
            f = ps.tile([128, 512], F32, tag=("st" if e % 2 else "outT"),
                        bufs=2, name=f"fin{e}")
            nc.tensor.matmul(f, pairN[0][:, e * 128:(e + 1) * 128], woTb[0],
                             start=True, stop=False)
            nc.tensor.matmul(f, pairN[1][:, e * 128:(e + 1) * 128], woTb[1],
                             start=False, stop=True)
            fo = work.tile([128, 512], F32, tag="fo", bufs=4, name=f"fo{e}")
            nc.vector.tensor_copy(fo, f)
            q = nc.gpsimd if e % 2 == 0 else nc.sync
            q.dma_start(out=out_d[e * 128:(e + 1) * 128, :], in_=fo)

    nc.compile()
    return nc


def _get_nc():
    global _CACHED_NC
    if _CACHED_NC is None:
        _CACHED_NC = _build_nc()
    return _CACHED_NC


def _make_in_maps(queries, keys, incidence_matrix, Wq, bq, Wk, bk, Wv, bv, Wo, bo):
    """Host-side sharding + layout marshalling (transposes + bf16 casts)."""
    queries = np.asarray(queries, dtype=np.float32)
    keys = np.asarray(keys, dtype=np.float32)
    incidence = np.asarray(incidence_matrix, dtype=np.float32)
    Wq = np.asarray(Wq, dtype=np.float32)
    Wk = np.asarray(Wk, dtype=np.float32)
    Wv = np.asarray(Wv, dtype=np.float32)
    Wo = np.asarray(Wo, dtype=np.float32)
    bq = np.asarray(bq, dtype=np.float32)
    bk = np.asarray(bk, dtype=np.float32)

    per_batch = {}
    for b in range(BS):
        qT = np.ascontiguousarray(
            queries[b].T.reshape(4, 128, E).transpose(1, 0, 2).reshape(
                128, 4096))
        kT = np.ascontiguousarray(keys[b].T)
        kTw = np.ascontiguousarray(
            kT.reshape(4, 128, 8, 512).transpose(2, 1, 0, 3).reshape(
                8, 128, 2048))
        mB = np.ascontiguousarray(
            incidence[b].T.reshape(NCHUNK // 2, 2, 128, E).transpose(
                0, 2, 1, 3).reshape(NCHUNK // 2, 128, 2 * E))
        per_batch[b] = (qT.astype(BF16NP), kTw.astype(BF16NP),
                        mB.astype(BF16NP))

    in_maps = []
    for core in range(8):
        b, g = core // 2, core % 2
        sl = slice(g * 256, (g + 1) * 256)
        qT8, kTw8, mB8 = per_batch[b]
        wAll = np.zeros((128, 4096), np.float32)
        for c in range(4):
            wAll[:, c * 256:(c + 1) * 256] = Wq[sl, :].T[c * 128:(c + 1) * 128]
            wAll[:, 1024 + c * 256:1024 + (c + 1) * 256] = \
                Wk[sl, :].T[c * 128:(c + 1) * 128]
            wAll[:, 2048 + c * 256:2048 + (c + 1) * 256] = \
                Wv[sl, :].T[c * 128:(c + 1) * 128]
        woT = Wo[:, sl].T
        wAll[:, 3072:3584] = woT[0:128]
        wAll[:, 3584:4096] = woT[128:256]
        wAll = wAll.astype(BF16NP)
        bias4 = np.stack([bq[sl][0:128], bq[sl][128:256],
                          bk[sl][0:128], bk[sl][128:256]], axis=1).astype(
            np.float32).copy()
        in_maps.append({
            "qT": qT8,
            "kTw": kTw8,
            "mB": mB8,
            "wAll": wAll,
            "bias4": bias4,
        })
    return in_maps


def kernel(**inputs):
    global LAST_EXEC_TIME_NS
    nc = _get_nc()
    in_maps = _make_in_maps(**inputs)
    trace = bool(os.environ.get("BASS_TRACE"))
    if trace:
        _install_ntff_hook()
    res = run_bass_kernel_spmd(nc, in_maps, core_ids=list(range(8)), trace=trace)
    LAST_EXEC_TIME_NS = res.exec_time_ns
    out = np.zeros((BS, E, D), np.float32)
    # attention rows sum to 1, so the V bias contributes bv @ Wo.T exactly;
    # add it (and bo) once here instead of on the device
    bias = (np.asarray(inputs["bo"], np.float32)
            + np.asarray(inputs["bv"], np.float32)
            @ np.asarray(inputs["Wo"], np.float32).T)
    for b in range(BS):
        out[b] = res.results[2 * b]["out"] + res.results[2 * b + 1]["out"] + bias
    return out


def _install_ntff_hook():
    """Recreate the missing antenv.axon_hooks glue so trace=True captures NTFF."""
    import types
    if "antenv.axon_hooks" in sys.modules:
        return
    try:
        from trn_agent_boot.trn_boot import _ntff_profile_via_ctypes
        hook = _ntff_profile_via_ctypes("/opt/axon/libaxon_pjrt.so")
        m = types.ModuleType("antenv.axon_hooks")
        m.get_axon_ntff_profile_hook = lambda: hook
        m.set_axon_ntff_profile_hook = lambda h: None
        sys.modules["antenv.axon_hooks"] = m
    except Exception:
        pass


# revision 16
# speedup vs baseline: 1.0423x; 1.0423x over previous
"""Trainium2 Bass kernel for MultiHeadHypergraphAttention.

Problem: queries (4, 1024, 512), keys (4, 4096, 512), incidence (4, 1024, 4096) i32,
torch-Linear Q/K/V/O projections, per-head masked softmax attention.

Sharding (8 cores): batch (4) x head-group (2 groups of 4 heads).
Core c handles batch b = c//2, head group g = c%2 and produces the partial
output projection for its 4 heads; the host sums the two partials per batch.

Device-side layout ("scores transposed"): S^T is computed with nodes on
partitions and edges on the free axis, so the incidence mask (host-marshalled
to (nodes, edges) bf16) is applied in its natural layout as a DVE multiply
of exp(s/8), and attention weights P^T feed the attn@V matmul directly as
the moving operand (V' stationary), producing O^T with head dims on
partitions - exactly the orientation the output projection needs.

Softmax normalization is folded into the output: V is augmented with a
ones-column so attn@V also produces row sums; O^T rows are divided by those
sums via a fast approximate reciprocal and a single-DRAM-bounce partition
broadcast. Masked entries are exact zeros (bf16 mask multiply).

All inputs arrive pre-cast from the host (bf16 activations/weights/mask),
so no on-chip dtype conversion of inputs is needed and HBM traffic drops
from ~30 MB (f32/i32) to ~13.5 MB per core. All matmuls run bf16 with f32
PSUM accumulation; the per-head scores matmul contracts over the full 128
partitions via zero-padded head-pair Q^T tiles. The engine balance is
ACT(exp)-bound: 128 exps of [128, 1024] ~ 141 us busy; PE ~ 119 us;
DVE ~ 120 us; DMA ~ 41 us. Heads run sequentially; each head's softmax
normalization hides inside the next head's stream.
"""

import sys
import os

for _p in ("/opt/trn_rl_repo",):
    if _p not in sys.path and os.path.isdir(_p):
        sys.path.insert(0, _p)

import numpy as np
import ml_dtypes
from contextlib import ExitStack

import concourse.bass as bass
import concourse.mybir as mybir
import concourse.tile as tile
from concourse import bacc
from concourse.bass_utils import run_bass_kernel_spmd

BF16 = mybir.dt.bfloat16
F32 = mybir.dt.float32

BF16NP = np.dtype(ml_dtypes.bfloat16)

BS, E, N, D = 4, 1024, 4096, 512
HL = 4                   # heads per core (local)
NCHUNK = N // 128        # 32 node chunks
ECHUNK = E // 128        # 8

LAST_EXEC_TIME_NS = None
_CACHED_NC = None


def _build_nc():
    nc = bacc.Bacc("TRN2", target_bir_lowering=False, debug=False, num_devices=8)

    qT_d = nc.dram_tensor("qT", (128, 4096), BF16, kind="ExternalInput").ap()
    kTw_d = nc.dram_tensor("kTw", (8, 128, 2048), BF16, kind="ExternalInput").ap()
    mB_d = nc.dram_tensor("mB", (NCHUNK // 2, 128, 2 * E), BF16, kind="ExternalInput").ap()
    # all projection weights packed [128, 4096] so the weight load is one
    # fat-line transfer: cols = wq(4x256) | wk(4x256) | wv(4x256) | wo(2x512)
    wAll_d = nc.dram_tensor("wAll", (128, 4096), BF16, kind="ExternalInput").ap()
    bias_d = nc.dram_tensor("bias4", (128, 4), F32, kind="ExternalInput").ap()
    out_d = nc.dram_tensor("out", (E, 512), F32, kind="ExternalOutput").ap()

    with tile.TileContext(nc) as tc, ExitStack() as ctx:
        persist = ctx.enter_context(tc.tile_pool(name="persist", bufs=1))
        work = ctx.enter_context(tc.tile_pool(name="work", bufs=1))
        ps = ctx.enter_context(tc.tile_pool(name="ps", bufs=1, space="PSUM"))
        dpool = ctx.enter_context(tc.tile_pool(name="dpool", bufs=1, space="DRAM"))

        # ---------------- constants ----------------
        QTs = [persist.tile([128, E], BF16, tag=f"QTs{l}", name=f"QTs{l}")
               for l in range(HL)]
        for l in range(HL):
            r = l % 2
            zsl = slice(64 * (1 - r), 64 * (1 - r) + 64)
            nc.vector.memset(QTs[l][zsl, :], 0.0)
        # V' bf16: [128 nodes, chunk, head, 65] ; col 64 = ones (row sums)
        Vs = persist.tile([128, NCHUNK * HL * 65], BF16, tag="Vs")
        Vs4 = Vs.rearrange("p (n h c) -> p n h c", n=NCHUNK, h=HL)
        nc.vector.memset(Vs4[:, :, :, 64:65], 1.0)

        # ------------- weight loads: one fat transfer each ----------------
        wAll = persist.tile([128, 4096], BF16, tag="wAll")
        nc.sync.dma_start(out=wAll, in_=wAll_d)
        biasT = persist.tile([128, 4], F32, tag="bias4")
        nc.gpsimd.dma_start(out=biasT, in_=bias_d)
        wqTb = [wAll[:, c * 256:(c + 1) * 256] for c in range(4)]
        wkTb = [wAll[:, 1024 + c * 256:1024 + (c + 1) * 256] for c in range(4)]
        wvTb = [wAll[:, 2048 + c * 256:2048 + (c + 1) * 256] for c in range(4)]
        woTb = [wAll[:, 3072 + p * 512:3072 + (p + 1) * 512] for p in range(2)]
        bqs = [biasT[:, p:p + 1] for p in range(2)]
        bks = [biasT[:, 2 + p:3 + p] for p in range(2)]

        # ------------- streaming input loads ------------------------------
        # mask chunks stream on the SWDGE queue (issued first so chunk 0
        # lands as early as possible); the HWDGE queue carries window 0,
        # then qT, then windows 1-7. Each kT window is one 512KB transfer
        # with 4KB contiguous lines (the four 128-row D-blocks side by
        # side in the free axis) so the DGE uses fat packets.
        Mb = persist.tile([128, NCHUNK * E], BF16, tag="Mb")
        kWins = [persist.tile([128, 2048], BF16, tag=f"kW{w}", name=f"kW{w}")
                 for w in range(8)]
        qAll = persist.tile([128, 4096], BF16, tag="qAll")
        qTb = [qAll[:, c * E:(c + 1) * E] for c in range(4)]
        nc.gpsimd.dma_start(out=qAll[:, 2 * E:], in_=qT_d[:, 2 * E:])
        for n in range(NCHUNK // 2):
            nc.gpsimd.dma_start(out=Mb[:, n * 2 * E:(n + 1) * 2 * E],
                                in_=mB_d[n])
        nc.sync.dma_start(out=qAll[:, 0:2 * E], in_=qT_d[:, 0:2 * E])
        for w in range(8):
            nc.sync.dma_start(out=kWins[w], in_=kTw_d[w])

        # ---------------- Q projection ----------------
        # QTs[l] (128, 1024) bf16: rows [64r, 64r+64) = head l's Q^T, rest 0
        # (l = 2p + r), so scores matmuls contract over the full 128
        # partitions against KTs[p]. Pair 1's Q/K projections are deferred
        # into the ACT-bound head-1/2 streams (only heads 2,3 need them).
        def proj_q(p):
            qp = ps.tile([128, E], F32, tag="st", bufs=2, name=f"qp{p}")
            for c in range(4):
                for e2 in range(2):
                    nc.tensor.matmul(
                        qp[:, e2 * 512:(e2 + 1) * 512],
                        wqTb[c][:, p * 128:(p + 1) * 128],
                        qTb[c][:, e2 * 512:(e2 + 1) * 512],
                        start=(c == 0), stop=(c == 3))
            for r in range(2):
                sl = slice(64 * r, 64 * r + 64)
                nc.vector.tensor_scalar_add(QTs[2 * p + r][sl, :], qp[sl, :],
                                            bqs[p][sl, :])

        proj_q(0)

        # ------------- K/V projections -----------------------------------
        KTs = [persist.tile([128, N], BF16, tag=f"KTs{p}", name=f"KTs{p}")
               for p in range(2)]
        pairN = [persist.tile([128, E], BF16, tag=f"pairN{p}", name=f"pairN{p}")
                 for p in range(2)]

        def proj_k(w, p):
            kp = ps.tile([128, 512], F32, tag="st", bufs=2, name=f"kp{p}_{w}")
            for c in range(4):
                nc.tensor.matmul(
                    kp, wkTb[c][:, p * 128:(p + 1) * 128],
                    kWins[w][:, c * 512:(c + 1) * 512],
                    start=(c == 0), stop=(c == 3))
            nc.vector.tensor_scalar_add(
                KTs[p][:, w * 512:(w + 1) * 512], kp, bks[p])

        def proj_v(n):
            w, j = divmod(n, 4)
            vp = ps.tile([128, 256], F32, tag="st", bufs=2, name=f"vp{n}")
            for c in range(4):
                blk = kWins[w][:, c * 512 + j * 128:c * 512 + j * 128 + 128]
                nc.tensor.matmul(vp, blk,
                                 wvTb[c], start=(c == 0), stop=(c == 3))
            dst = Vs4[:, n, :, 0:64]
            src = vp.rearrange("p (h c) -> p h c", h=4)
            nc.vector.tensor_copy(dst, src)

        # ------------- attention helpers ---------------------------------
        oTs = {}
        Ps = {}

        def score_part(l, n):
            # scores + exp + mask for (head l, node chunk n) -> P^T in Ps
            p = l // 2
            st = ps.tile([128, E], F32, tag="st", bufs=2, name=f"st{l}_{n}")
            kblk = KTs[p][:, n * 128:(n + 1) * 128]
            for e2 in range(2):
                sl = slice(e2 * 512, (e2 + 1) * 512)
                nc.tensor.matmul(st[:, sl], kblk, QTs[l][:, sl],
                                 start=True, stop=True)
            Praw = work.tile([128, E], BF16, tag="Praw", bufs=6,
                             name=f"Praw{l}_{n}")
            nc.scalar.activation(Praw, st, mybir.ActivationFunctionType.Exp,
                                 bias=0.0, scale=0.125)
            P = work.tile([128, E], BF16, tag="P", bufs=6, name=f"P{l}_{n}")
            nc.vector.tensor_mul(P, Praw, Mb[:, n * E:(n + 1) * E])
            Ps[(l, n)] = P

        def av_part(l, n):
            # attn @ V' for (head l, node chunk n), accumulating into oTs[l]
            P = Ps.pop((l, n))
            vblk = Vs4[:, n, l]
            for e2 in range(2):
                sl = slice(e2 * 512, (e2 + 1) * 512)
                nc.tensor.matmul(oTs[l][:, sl], vblk, P[:, sl],
                                 start=(n == 0), stop=(n == NCHUNK - 1))

        def head_seq(l):
            seq = []
            for n in range(NCHUNK):
                seq.append(lambda l=l, n=n: score_part(l, n))
                if n > 0:
                    seq.append(lambda l=l, n=n - 1: av_part(l, n))
            seq.append(lambda l=l: av_part(l, NCHUNK - 1))
            return seq

        # ------------- normalization (via DRAM bounces, as baseline) -----
        norm_state = {}

        def norm_stage1(l):
            # copy the exp-sum row out of PSUM, bounce to DRAM and back
            # reshaped (64, 16) so the reciprocal runs 64 lanes wide
            sums = work.tile([1, E], F32, tag="sums", bufs=2, name=f"sums{l}")
            nc.vector.tensor_copy(sums, oTs[l][64:65, :])
            sums_d = dpool.tile([1, E], F32, tag="sums_d", bufs=2,
                                name=f"sums_d{l}")
            nc.sync.dma_start(out=sums_d, in_=sums)
            sums64 = work.tile([64, 16], F32, tag="sums64", bufs=2,
                               name=f"sums64{l}")
            nc.sync.dma_start(
                out=sums64, in_=sums_d.rearrange("one (p k) -> (one p) k", p=64))
            norm_state[l] = sums64

        def norm_stage2(l):
            sums64 = norm_state.pop(l)
            recip64 = work.tile([64, 16], F32, tag="recip64", bufs=2,
                                name=f"recip64{l}")
            nc.vector.reciprocal(recip64, sums64)
            rec_d = dpool.tile([64, 16], F32, tag="rec_d", bufs=2,
                               name=f"rec_d{l}")
            nc.sync.dma_start(out=rec_d, in_=recip64)
            norm_state[l] = rec_d

        def norm_stage3(l):
            rec_row = norm_state[l].rearrange("p k -> (p k)").unsqueeze(0)
            recb = work.tile([64, E], F32, tag="recb", bufs=2, name=f"recb{l}")
            nc.sync.dma_start(out=recb, in_=rec_row.to_broadcast((64, E)))
            norm_state[l] = recb

        def norm_stage4(l):
            p, r = l // 2, l % 2
            recb = norm_state.pop(l)
            nc.vector.tensor_mul(pairN[p][64 * r:64 * r + 64, :],
                                 oTs[l][0:64, :], recb)

        NORM_STAGES = (norm_stage1, norm_stage2, norm_stage3, norm_stage4)

        def normalize(l):
            for s in NORM_STAGES:
                s(l)

        # ------------- merged pipeline -----------------------------------
        for l in (0, 1):
            oTs[l] = ps.tile([65, E], F32, tag="outT", bufs=2, name=f"oT{l}")

        # head 0 trails the K/V projection windows by one window
        proj_q(1)
        h0 = head_seq(0)
        h0i = 0
        for w in range(8):
            steps = [lambda w=w: proj_k(w, 0), lambda w=w: proj_k(w, 1)] + \
                    [lambda n=n: proj_v(n) for n in range(4 * w, 4 * w + 4)]
            for i, step in enumerate(steps):
                if w > 0 and i < 5 and h0i < len(h0):
                    h0[h0i]()
                    h0i += 1
                step()
        while h0i < len(h0):
            h0[h0i]()
            h0i += 1

        # heads 1-3; head l-1's normalization stages woven into head l's
        # stream so their latency hides
        for l in (1, 2, 3):
            if l >= 2:
                oTs[l] = ps.tile([65, E], F32, tag="outT", bufs=2,
                                 name=f"oT{l}")
            stages = {2: 0, 8: 1, 14: 2, 20: 3}
            for idx, item in enumerate(head_seq(l)):
                item()
                if idx in stages:
                    NORM_STAGES[stages[idx]](l - 1)
        normalize(3)

        # ---------------- output projection (partial) --------------------
        # fin tiles rotate over both PSUM tags (4 slots) and the result is
        # DMAed straight from PSUM, alternating queues
        for e in range(ECHUNK):
            f = ps.tile([128, 512], F32, tag=("st" if e % 2 else "outT"),
                        bufs=2, name=f"fin{e}")
            nc.tensor.matmul(f, pairN[0][:, e * 128:(e + 1) * 128], woTb[0],
                             start=True, stop=False)
            nc.tensor.matmul(f, pairN[1][:, e * 128:(e + 1) * 128], woTb[1],
                             start=False, stop=True)
            fo = work.tile([128, 512], F32, tag="fo", bufs=4, name=f"fo{e}")
            nc.vector.tensor_copy(fo, f)
            q = nc.gpsimd if e % 2 == 0 else nc.sync
            q.dma_start(out=out_d[e * 128:(e + 1) * 128, :], in_=fo)

    nc.compile()
    return nc


def _get_nc():
    global _CACHED_NC
    if _CACHED_NC is None:
        _CACHED_NC = _build_nc()
    return _CACHED_NC


def _make_in_maps(queries, keys, incidence_matrix, Wq, bq, Wk, bk, Wv, bv, Wo, bo):
    """Host-side sharding + layout marshalling (transposes + bf16 casts)."""
    queries = np.asarray(queries, dtype=np.float32)
    keys = np.asarray(keys, dtype=np.float32)
    incidence = np.asarray(incidence_matrix, dtype=np.float32)
    Wq = np.asarray(Wq, dtype=np.float32)
    Wk = np.asarray(Wk, dtype=np.float32)
    Wv = np.asarray(Wv, dtype=np.float32)
    Wo = np.asarray(Wo, dtype=np.float32)
    bq = np.asarray(bq, dtype=np.float32)
    bk = np.asarray(bk, dtype=np.float32)

    per_batch = {}
    for b in range(BS):
        qT = np.ascontiguousarray(
            queries[b].T.reshape(4, 128, E).transpose(1, 0, 2).reshape(
                128, 4096))
        kT = np.ascontiguousarray(keys[b].T)
        kTw = np.ascontiguousarray(
            kT.reshape(4, 128, 8, 512).transpose(2, 1, 0, 3).reshape(
                8, 128, 2048))
        mB = np.ascontiguousarray(
            incidence[b].T.reshape(NCHUNK // 2, 2, 128, E).transpose(
                0, 2, 1, 3).reshape(NCHUNK // 2, 128, 2 * E))
        per_batch[b] = (qT.astype(BF16NP), kTw.astype(BF16NP),
                        mB.astype(BF16NP))

    in_maps = []
    for core in range(8):
        b, g = core // 2, core % 2
        sl = slice(g * 256, (g + 1) * 256)
        qT8, kTw8, mB8 = per_batch[b]
        wAll = np.zeros((128, 4096), np.float32)
        for c in range(4):
            wAll[:, c * 256:(c + 1) * 256] = Wq[sl, :].T[c * 128:(c + 1) * 128]
            wAll[:, 1024 + c * 256:1024 + (c + 1) * 256] = \
                Wk[sl, :].T[c * 128:(c + 1) * 128]
            wAll[:, 2048 + c * 256:2048 + (c + 1) * 256] = \
                Wv[sl, :].T[c * 128:(c + 1) * 128]
        woT = Wo[:, sl].T
        wAll[:, 3072:3584] = woT[0:128]
        wAll[:, 3584:4096] = woT[128:256]
        wAll = wAll.astype(BF16NP)
        bias4 = np.stack([bq[sl][0:128], bq[sl][128:256],
                          bk[sl][0:128], bk[sl][128:256]], axis=1).astype(
            np.float32).copy()
        in_maps.append({
            "qT": qT8,
            "kTw": kTw8,
            "mB": mB8,
            "wAll": wAll,
            "bias4": bias4,
        })
    return in_maps


def kernel(**inputs):
    global LAST_EXEC_TIME_NS
    nc = _get_nc()
    in_maps = _make_in_maps(**inputs)
    trace = bool(os.environ.get("BASS_TRACE"))
    if trace:
        _install_ntff_hook()
    res = run_bass_kernel_spmd(nc, in_maps, core_ids=list(range(8)), trace=trace)
    LAST_EXEC_TIME_NS = res.exec_time_ns
    out = np.zeros((BS, E, D), np.float32)
    # attention rows sum to 1, so the V bias contributes bv @ Wo.T exactly;
    # add it (and bo) once here instead of on the device
    bias = (np.asarray(inputs["bo"], np.float32)
            + np.asarray(inputs["bv"], np.float32)
            @ np.asarray(inputs["Wo"], np.float32).T)
    for b in range(BS):
        out[b] = res.results[2 * b]["out"] + res.results[2 * b + 1]["out"] + bias
    return out


def _install_ntff_hook():
    """Recreate the missing antenv.axon_hooks glue so trace=True captures NTFF."""
    import types
    if "antenv.axon_hooks" in sys.modules:
        return
    try:
        from trn_agent_boot.trn_boot import _ntff_profile_via_ctypes
        hook = _ntff_profile_via_ctypes("/opt/axon/libaxon_pjrt.so")
        m = types.ModuleType("antenv.axon_hooks")
        m.get_axon_ntff_profile_hook = lambda: hook
        m.set_axon_ntff_profile_hook = lambda h: None
        sys.modules["antenv.axon_hooks"] = m
    except Exception:
        pass


# revision 17
# speedup vs baseline: 1.0484x; 1.0059x over previous
"""Trainium2 Bass kernel for MultiHeadHypergraphAttention.

Problem: queries (4, 1024, 512), keys (4, 4096, 512), incidence (4, 1024, 4096) i32,
torch-Linear Q/K/V/O projections, per-head masked softmax attention.

Sharding (8 cores): batch (4) x head-group (2 groups of 4 heads).
Core c handles batch b = c//2, head group g = c%2 and produces the partial
output projection for its 4 heads; the host sums the two partials per batch.

Device-side layout ("scores transposed"): S^T is computed with nodes on
partitions and edges on the free axis, so the incidence mask (host-marshalled
to (nodes, edges) bf16) is applied in its natural layout as a DVE multiply
of exp(s/8), and attention weights P^T feed the attn@V matmul directly as
the moving operand (V' stationary), producing O^T with head dims on
partitions - exactly the orientation the output projection needs.

Softmax normalization is folded into the output: V is augmented with a
ones-column so attn@V also produces row sums; O^T rows are divided by those
sums via a fast approximate reciprocal and a single-DRAM-bounce partition
broadcast. Masked entries are exact zeros (bf16 mask multiply).

All inputs arrive pre-cast from the host (bf16 activations/weights/mask),
so no on-chip dtype conversion of inputs is needed and HBM traffic drops
from ~30 MB (f32/i32) to ~13.5 MB per core. All matmuls run bf16 with f32
PSUM accumulation; the per-head scores matmul contracts over the full 128
partitions via zero-padded head-pair Q^T tiles. The engine balance is
ACT(exp)-bound: 128 exps of [128, 1024] ~ 141 us busy; PE ~ 119 us;
DVE ~ 120 us; DMA ~ 41 us. Heads run sequentially; each head's softmax
normalization hides inside the next head's stream.
"""

import sys
import os

for _p in ("/opt/trn_rl_repo",):
    if _p not in sys.path and os.path.isdir(_p):
        sys.path.insert(0, _p)

import numpy as np
import ml_dtypes
from contextlib import ExitStack

import concourse.bass as bass
import concourse.mybir as mybir
import concourse.tile as tile
from concourse import bacc
from concourse.bass_utils import run_bass_kernel_spmd

BF16 = mybir.dt.bfloat16
F32 = mybir.dt.float32

BF16NP = np.dtype(ml_dtypes.bfloat16)

BS, E, N, D = 4, 1024, 4096, 512
HL = 4                   # heads per core (local)
NCHUNK = N // 128        # 32 node chunks
ECHUNK = E // 128        # 8

LAST_EXEC_TIME_NS = None
_CACHED_NC = None


def _build_nc():
    nc = bacc.Bacc("TRN2", target_bir_lowering=False, debug=False, num_devices=8)

    qT_d = nc.dram_tensor("qT", (128, 4096), BF16, kind="ExternalInput").ap()
    kTw_d = nc.dram_tensor("kTw", (8, 128, 2048), BF16, kind="ExternalInput").ap()
    mB_d = nc.dram_tensor("mB", (NCHUNK // 2, 128, 2 * E), BF16, kind="ExternalInput").ap()
    # all projection weights packed [128, 4096] so the weight load is one
    # fat-line transfer: cols = wq(4x256) | wk(4x256) | wv(4x256) | wo(2x512)
    wAll_d = nc.dram_tensor("wAll", (128, 4096), BF16, kind="ExternalInput").ap()
    bias_d = nc.dram_tensor("bias4", (128, 4), F32, kind="ExternalInput").ap()
    out_d = nc.dram_tensor("out", (E, 512), F32, kind="ExternalOutput").ap()

    with tile.TileContext(nc) as tc, ExitStack() as ctx:
        persist = ctx.enter_context(tc.tile_pool(name="persist", bufs=1))
        work = ctx.enter_context(tc.tile_pool(name="work", bufs=1))
        ps = ctx.enter_context(tc.tile_pool(name="ps", bufs=1, space="PSUM"))
        dpool = ctx.enter_context(tc.tile_pool(name="dpool", bufs=1, space="DRAM"))

        # ---------------- constants ----------------
        QTs = [persist.tile([128, E], BF16, tag=f"QTs{l}", name=f"QTs{l}")
               for l in range(HL)]
        for l in range(HL):
            r = l % 2
            zsl = slice(64 * (1 - r), 64 * (1 - r) + 64)
            nc.vector.memset(QTs[l][zsl, :], 0.0)
        # V' bf16: [128 nodes, chunk, head, 65] ; col 64 = ones (row sums)
        Vs = persist.tile([128, NCHUNK * HL * 65], BF16, tag="Vs")
        Vs4 = Vs.rearrange("p (n h c) -> p n h c", n=NCHUNK, h=HL)
        nc.vector.memset(Vs4[:, :, :, 64:65], 1.0)

        # ------------- weight loads: one fat transfer each ----------------
        wAll = persist.tile([128, 4096], BF16, tag="wAll")
        nc.sync.dma_start(out=wAll, in_=wAll_d)
        biasT = persist.tile([128, 4], F32, tag="bias4")
        nc.gpsimd.dma_start(out=biasT, in_=bias_d)
        wqTb = [wAll[:, c * 256:(c + 1) * 256] for c in range(4)]
        wkTb = [wAll[:, 1024 + c * 256:1024 + (c + 1) * 256] for c in range(4)]
        wvTb = [wAll[:, 2048 + c * 256:2048 + (c + 1) * 256] for c in range(4)]
        woTb = [wAll[:, 3072 + p * 512:3072 + (p + 1) * 512] for p in range(2)]
        bqs = [biasT[:, p:p + 1] for p in range(2)]
        bks = [biasT[:, 2 + p:3 + p] for p in range(2)]

        # ------------- streaming input loads ------------------------------
        # mask chunks stream on the SWDGE queue (issued first so chunk 0
        # lands as early as possible); the HWDGE queue carries window 0,
        # then qT, then windows 1-7. Each kT window is one 512KB transfer
        # with 4KB contiguous lines (the four 128-row D-blocks side by
        # side in the free axis) so the DGE uses fat packets.
        Mb = persist.tile([128, NCHUNK * E], BF16, tag="Mb")
        kWins = [persist.tile([128, 2048], BF16, tag=f"kW{w}", name=f"kW{w}")
                 for w in range(8)]
        qAll = persist.tile([128, 4096], BF16, tag="qAll")
        qTb = [qAll[:, c * E:(c + 1) * E] for c in range(4)]
        nc.gpsimd.dma_start(out=qAll[:, 2 * E:], in_=qT_d[:, 2 * E:])
        for n in range(NCHUNK // 2):
            nc.gpsimd.dma_start(out=Mb[:, n * 2 * E:(n + 1) * 2 * E],
                                in_=mB_d[n])
        nc.sync.dma_start(out=qAll[:, 0:2 * E], in_=qT_d[:, 0:2 * E])
        for w in range(8):
            nc.sync.dma_start(out=kWins[w], in_=kTw_d[w])

        # ---------------- Q projection ----------------
        # QTs[l] (128, 1024) bf16: rows [64r, 64r+64) = head l's Q^T, rest 0
        # (l = 2p + r), so scores matmuls contract over the full 128
        # partitions against KTs[p]. Pair 1's Q/K projections are deferred
        # into the ACT-bound head-1/2 streams (only heads 2,3 need them).
        def proj_q(p):
            qp = ps.tile([128, E], F32, tag="st", bufs=2, name=f"qp{p}")
            for c in range(4):
                for e2 in range(2):
                    nc.tensor.matmul(
                        qp[:, e2 * 512:(e2 + 1) * 512],
                        wqTb[c][:, p * 128:(p + 1) * 128],
                        qTb[c][:, e2 * 512:(e2 + 1) * 512],
                        start=(c == 0), stop=(c == 3))
            for r in range(2):
                sl = slice(64 * r, 64 * r + 64)
                nc.vector.tensor_scalar_add(QTs[2 * p + r][sl, :], qp[sl, :],
                                            bqs[p][sl, :])

        proj_q(0)

        # ------------- K/V projections -----------------------------------
        KTs = [persist.tile([128, N], BF16, tag=f"KTs{p}", name=f"KTs{p}")
               for p in range(2)]
        pairN = [persist.tile([128, E], BF16, tag=f"pairN{p}", name=f"pairN{p}")
                 for p in range(2)]

        def proj_k(w, p):
            kp = ps.tile([128, 512], F32, tag="st", bufs=2, name=f"kp{p}_{w}")
            for c in range(4):
                nc.tensor.matmul(
                    kp, wkTb[c][:, p * 128:(p + 1) * 128],
                    kWins[w][:, c * 512:(c + 1) * 512],
                    start=(c == 0), stop=(c == 3))
            nc.vector.tensor_scalar_add(
                KTs[p][:, w * 512:(w + 1) * 512], kp, bks[p])

        def proj_v(n):
            w, j = divmod(n, 4)
            vp = ps.tile([128, 256], F32, tag="st", bufs=2, name=f"vp{n}")
            for c in range(4):
                blk = kWins[w][:, c * 512 + j * 128:c * 512 + j * 128 + 128]
                nc.tensor.matmul(vp, blk,
                                 wvTb[c], start=(c == 0), stop=(c == 3))
            dst = Vs4[:, n, :, 0:64]
            src = vp.rearrange("p (h c) -> p h c", h=4)
            nc.vector.tensor_copy(dst, src)

        # ------------- attention helpers ---------------------------------
        oTs = {}
        Ps = {}

        def score_part(l, n):
            # scores + exp + mask for (head l, node chunk n) -> P^T in Ps
            p = l // 2
            st = ps.tile([128, E], F32, tag="st", bufs=2, name=f"st{l}_{n}")
            kblk = KTs[p][:, n * 128:(n + 1) * 128]
            for e2 in range(2):
                sl = slice(e2 * 512, (e2 + 1) * 512)
                nc.tensor.matmul(st[:, sl], kblk, QTs[l][:, sl],
                                 start=True, stop=True)
            Praw = work.tile([128, E], BF16, tag="Praw", bufs=6,
                             name=f"Praw{l}_{n}")
            nc.scalar.activation(Praw, st, mybir.ActivationFunctionType.Exp,
                                 bias=0.0, scale=0.125)
            P = work.tile([128, E], BF16, tag="P", bufs=6, name=f"P{l}_{n}")
            nc.vector.tensor_mul(P, Praw, Mb[:, n * E:(n + 1) * E])
            Ps[(l, n)] = P

        def av_part(l, n):
            # attn @ V' for (head l, node chunk n), accumulating into oTs[l]
            P = Ps.pop((l, n))
            vblk = Vs4[:, n, l]
            for e2 in range(2):
                sl = slice(e2 * 512, (e2 + 1) * 512)
                nc.tensor.matmul(oTs[l][:, sl], vblk, P[:, sl],
                                 start=(n == 0), stop=(n == NCHUNK - 1))

        def head_seq(l):
            seq = []
            for n in range(NCHUNK):
                seq.append(lambda l=l, n=n: score_part(l, n))
                if n > 0:
                    seq.append(lambda l=l, n=n - 1: av_part(l, n))
            seq.append(lambda l=l: av_part(l, NCHUNK - 1))
            return seq

        # ------------- normalization (via DRAM bounces, as baseline) -----
        norm_state = {}

        def norm_stage1(l):
            # copy the exp-sum row out of PSUM, bounce to DRAM and back
            # reshaped (64, 16) so the reciprocal runs 64 lanes wide
            sums = work.tile([1, E], F32, tag="sums", bufs=2, name=f"sums{l}")
            nc.vector.tensor_copy(sums, oTs[l][64:65, :])
            sums_d = dpool.tile([1, E], F32, tag="sums_d", bufs=2,
                                name=f"sums_d{l}")
            nc.sync.dma_start(out=sums_d, in_=sums)
            sums64 = work.tile([64, 16], F32, tag="sums64", bufs=2,
                               name=f"sums64{l}")
            nc.sync.dma_start(
                out=sums64, in_=sums_d.rearrange("one (p k) -> (one p) k", p=64))
            norm_state[l] = sums64

        def norm_stage2(l):
            sums64 = norm_state.pop(l)
            recip64 = work.tile([64, 16], F32, tag="recip64", bufs=2,
                                name=f"recip64{l}")
            nc.vector.reciprocal(recip64, sums64)
            rec_d = dpool.tile([64, 16], F32, tag="rec_d", bufs=2,
                               name=f"rec_d{l}")
            nc.sync.dma_start(out=rec_d, in_=recip64)
            norm_state[l] = rec_d

        def norm_stage3(l):
            rec_row = norm_state[l].rearrange("p k -> (p k)").unsqueeze(0)
            recb = work.tile([64, E], F32, tag="recb", bufs=2, name=f"recb{l}")
            nc.sync.dma_start(out=recb, in_=rec_row.to_broadcast((64, E)))
            norm_state[l] = recb

        def norm_stage4(l):
            p, r = l // 2, l % 2
            recb = norm_state.pop(l)
            nc.vector.tensor_mul(pairN[p][64 * r:64 * r + 64, :],
                                 oTs[l][0:64, :], recb)

        NORM_STAGES = (norm_stage1, norm_stage2, norm_stage3, norm_stage4)

        def normalize(l):
            for s in NORM_STAGES:
                s(l)

        # ------------- merged pipeline -----------------------------------
        for l in (0, 1):
            oTs[l] = ps.tile([65, E], F32, tag="outT", bufs=2, name=f"oT{l}")

        # head 0 trails the K/V projection windows by one window
        proj_q(1)
        h0 = head_seq(0)
        h0i = 0
        for w in range(8):
            steps = [lambda w=w: proj_k(w, 0), lambda w=w: proj_k(w, 1)] + \
                    [lambda n=n: proj_v(n) for n in range(4 * w, 4 * w + 4)]
            for i, step in enumerate(steps):
                if w > 0 and i < 5 and h0i < len(h0):
                    h0[h0i]()
                    h0i += 1
                step()
        while h0i < len(h0):
            h0[h0i]()
            h0i += 1

        # heads 1-3; head l-1's normalization stages woven into head l's
        # stream so their latency hides
        for l in (1, 2, 3):
            if l >= 2:
                oTs[l] = ps.tile([65, E], F32, tag="outT", bufs=2,
                                 name=f"oT{l}")
            stages = {2: 0, 8: 1, 14: 2, 20: 3}
            for idx, item in enumerate(head_seq(l)):
                item()
                if idx in stages:
                    NORM_STAGES[stages[idx]](l - 1)

        # ---------------- output projection (partial) --------------------
        # pair-0 halves for the first three chunks pre-issue into the free
        # PSUM slots while head 3's normalization chain is in flight; the
        # rest follow once pairN[1] is complete
        fin_tag = {0: "outT", 1: "st", 2: "st", 3: "outT",
                   4: "st", 5: "st", 6: "outT", 7: "st"}
        fins = {}
        for e in range(3):
            f = ps.tile([128, 512], F32, tag=fin_tag[e], bufs=2,
                        name=f"fin{e}")
            nc.tensor.matmul(f, pairN[0][:, e * 128:(e + 1) * 128], woTb[0],
                             start=True, stop=False)
            fins[e] = f
        normalize(3)
        for e in range(ECHUNK):
            if e in fins:
                f = fins[e]
            else:
                f = ps.tile([128, 512], F32, tag=fin_tag[e], bufs=2,
                            name=f"fin{e}")
                nc.tensor.matmul(f, pairN[0][:, e * 128:(e + 1) * 128],
                                 woTb[0], start=True, stop=False)
            nc.tensor.matmul(f, pairN[1][:, e * 128:(e + 1) * 128], woTb[1],
                             start=False, stop=True)
            fo = work.tile([128, 512], F32, tag="fo", bufs=4, name=f"fo{e}")
            nc.vector.tensor_copy(fo, f)
            nc.sync.dma_start(out=out_d[e * 128:(e + 1) * 128, :], in_=fo)

    nc.compile()
    return nc


def _get_nc():
    global _CACHED_NC
    if _CACHED_NC is None:
        _CACHED_NC = _build_nc()
    return _CACHED_NC


def _make_in_maps(queries, keys, incidence_matrix, Wq, bq, Wk, bk, Wv, bv, Wo, bo):
    """Host-side sharding + layout marshalling (transposes + bf16 casts)."""
    queries = np.asarray(queries, dtype=np.float32)
    keys = np.asarray(keys, dtype=np.float32)
    incidence = np.asarray(incidence_matrix, dtype=np.float32)
    Wq = np.asarray(Wq, dtype=np.float32)
    Wk = np.asarray(Wk, dtype=np.float32)
    Wv = np.asarray(Wv, dtype=np.float32)
    Wo = np.asarray(Wo, dtype=np.float32)
    bq = np.asarray(bq, dtype=np.float32)
    bk = np.asarray(bk, dtype=np.float32)

    per_batch = {}
    for b in range(BS):
        qT = np.ascontiguousarray(
            queries[b].T.reshape(4, 128, E).transpose(1, 0, 2).reshape(
                128, 4096))
        kT = np.ascontiguousarray(keys[b].T)
        kTw = np.ascontiguousarray(
            kT.reshape(4, 128, 8, 512).transpose(2, 1, 0, 3).reshape(
                8, 128, 2048))
        mB = np.ascontiguousarray(
            incidence[b].T.reshape(NCHUNK // 2, 2, 128, E).transpose(
                0, 2, 1, 3).reshape(NCHUNK // 2, 128, 2 * E))
        per_batch[b] = (qT.astype(BF16NP), kTw.astype(BF16NP),
                        mB.astype(BF16NP))

    in_maps = []
    for core in range(8):
        b, g = core // 2, core % 2
        sl = slice(g * 256, (g + 1) * 256)
        qT8, kTw8, mB8 = per_batch[b]
        wAll = np.zeros((128, 4096), np.float32)
        for c in range(4):
            wAll[:, c * 256:(c + 1) * 256] = Wq[sl, :].T[c * 128:(c + 1) * 128]
            wAll[:, 1024 + c * 256:1024 + (c + 1) * 256] = \
                Wk[sl, :].T[c * 128:(c + 1) * 128]
            wAll[:, 2048 + c * 256:2048 + (c + 1) * 256] = \
                Wv[sl, :].T[c * 128:(c + 1) * 128]
        woT = Wo[:, sl].T
        wAll[:, 3072:3584] = woT[0:128]
        wAll[:, 3584:4096] = woT[128:256]
        wAll = wAll.astype(BF16NP)
        bias4 = np.stack([bq[sl][0:128], bq[sl][128:256],
                          bk[sl][0:128], bk[sl][128:256]], axis=1).astype(
            np.float32).copy()
        in_maps.append({
            "qT": qT8,
            "kTw": kTw8,
            "mB": mB8,
            "wAll": wAll,
            "bias4": bias4,
        })
    return in_maps


def kernel(**inputs):
    global LAST_EXEC_TIME_NS
    nc = _get_nc()
    in_maps = _make_in_maps(**inputs)
    trace = bool(os.environ.get("BASS_TRACE"))
    if trace:
        _install_ntff_hook()
    res = run_bass_kernel_spmd(nc, in_maps, core_ids=list(range(8)), trace=trace)
    LAST_EXEC_TIME_NS = res.exec_time_ns
    out = np.zeros((BS, E, D), np.float32)
    # attention rows sum to 1, so the V bias contributes bv @ Wo.T exactly;
    # add it (and bo) once here instead of on the device
    bias = (np.asarray(inputs["bo"], np.float32)
            + np.asarray(inputs["bv"], np.float32)
            @ np.asarray(inputs["Wo"], np.float32).T)
    for b in range(BS):
        out[b] = res.results[2 * b]["out"] + res.results[2 * b + 1]["out"] + bias
    return out


def _install_ntff_hook():
    """Recreate the missing antenv.axon_hooks glue so trace=True captures NTFF."""
    import types
    if "antenv.axon_hooks" in sys.modules:
        return
    try:
        from trn_agent_boot.trn_boot import _ntff_profile_via_ctypes
        hook = _ntff_profile_via_ctypes("/opt/axon/libaxon_pjrt.so")
        m = types.ModuleType("antenv.axon_hooks")
        m.get_axon_ntff_profile_hook = lambda: hook
        m.set_axon_ntff_profile_hook = lambda h: None
        sys.modules["antenv.axon_hooks"] = m
    except Exception:
        pass


# revision 22
# speedup vs baseline: 1.0697x; 1.0203x over previous
"""Trainium2 Bass kernel for MultiHeadHypergraphAttention.

Problem: queries (4, 1024, 512), keys (4, 4096, 512), incidence (4, 1024, 4096) i32,
torch-Linear Q/K/V/O projections, per-head masked softmax attention.

Sharding (8 cores): batch (4) x head-group (2 groups of 4 heads).
Core c handles batch b = c//2, head group g = c%2 and produces the partial
output projection for its 4 heads; the host sums the two partials per batch.

Device-side layout ("scores transposed"): S^T is computed with nodes on
partitions and edges on the free axis, so the incidence mask (host-marshalled
to (nodes, edges) bf16) is applied in its natural layout as a DVE multiply
of exp(s/8), and attention weights P^T feed the attn@V matmul directly as
the moving operand (V' stationary), producing O^T with head dims on
partitions - exactly the orientation the output projection needs.

Softmax normalization is folded into the output: V is augmented with a
ones-column so attn@V also produces row sums; O^T rows are divided by those
sums via a fast approximate reciprocal and a single-DRAM-bounce partition
broadcast. Masked entries are exact zeros (bf16 mask multiply).

All inputs arrive pre-cast from the host (bf16 activations/weights/mask),
so no on-chip dtype conversion of inputs is needed and HBM traffic drops
from ~30 MB (f32/i32) to ~13.5 MB per core. All matmuls run bf16 with f32
PSUM accumulation; the per-head scores matmul contracts over the full 128
partitions via zero-padded head-pair Q^T tiles. The engine balance is
ACT(exp)-bound: 128 exps of [128, 1024] ~ 141 us busy; PE ~ 119 us;
DVE ~ 120 us; DMA ~ 41 us. Heads run sequentially; each head's softmax
normalization hides inside the next head's stream.
"""

import sys
import os

for _p in ("/opt/trn_rl_repo",):
    if _p not in sys.path and os.path.isdir(_p):
        sys.path.insert(0, _p)

import numpy as np
import ml_dtypes
from contextlib import ExitStack

import concourse.bass as bass
import concourse.mybir as mybir
import concourse.tile as tile
from concourse import bacc
from concourse.bass_utils import run_bass_kernel_spmd

BF16 = mybir.dt.bfloat16
F32 = mybir.dt.float32

BF16NP = np.dtype(ml_dtypes.bfloat16)

BS, E, N, D = 4, 1024, 4096, 512
HL = 4                   # heads per core (local)
NCHUNK = N // 128        # 32 node chunks
ECHUNK = E // 128        # 8

LAST_EXEC_TIME_NS = None
_CACHED_NC = None


def _build_nc():
    nc = bacc.Bacc("TRN2", target_bir_lowering=False, debug=False, num_devices=8)

    qT_d = nc.dram_tensor("qT", (128, 4096), BF16, kind="ExternalInput").ap()
    kTw_d = nc.dram_tensor("kTw", (8, 128, 2048), BF16, kind="ExternalInput").ap()
    mB_d = nc.dram_tensor("mB", (NCHUNK // 2, 128, 2 * E), BF16, kind="ExternalInput").ap()
    # all projection weights packed [128, 4096] so the weight load is one
    # fat-line transfer: cols = wq(4x256) | wk(4x256) | wv(4x256) | wo(2x512)
    wAll_d = nc.dram_tensor("wAll", (128, 4096), BF16, kind="ExternalInput").ap()
    bias_d = nc.dram_tensor("bias4", (128, 4), F32, kind="ExternalInput").ap()
    out_d = nc.dram_tensor("out", (E, 512), F32, kind="ExternalOutput").ap()

    with tile.TileContext(nc) as tc, ExitStack() as ctx:
        persist = ctx.enter_context(tc.tile_pool(name="persist", bufs=1))
        work = ctx.enter_context(tc.tile_pool(name="work", bufs=1))
        ps = ctx.enter_context(tc.tile_pool(name="ps", bufs=1, space="PSUM"))
        dpool = ctx.enter_context(tc.tile_pool(name="dpool", bufs=1, space="DRAM"))

        # ---------------- constants ----------------
        QTs = [persist.tile([128, E], BF16, tag=f"QTs{l}", name=f"QTs{l}")
               for l in range(HL)]
        for l in range(HL):
            r = l % 2
            zsl = slice(64 * (1 - r), 64 * (1 - r) + 64)
            nc.vector.memset(QTs[l][zsl, :], 0.0)
        # V' bf16: [128 nodes, chunk, head, 65] ; col 64 = ones (row sums)
        Vs = persist.tile([128, NCHUNK * HL * 65], BF16, tag="Vs")
        Vs4 = Vs.rearrange("p (n h c) -> p n h c", n=NCHUNK, h=HL)
        nc.vector.memset(Vs4[:, :, :, 64:65], 1.0)
        ones_f32 = persist.tile([1, 64], F32, tag="ones_f32")
        nc.vector.memset(ones_f32, 1.0)
        ones_row = persist.tile([1, 64], mybir.dt.float32r, tag="ones_row")
        nc.vector.tensor_copy(ones_row, ones_f32)

        # ------------- weight loads: one fat transfer each ----------------
        wAll = persist.tile([128, 4096], BF16, tag="wAll")
        nc.sync.dma_start(out=wAll, in_=wAll_d)
        biasT = persist.tile([128, 4], F32, tag="bias4")
        nc.gpsimd.dma_start(out=biasT, in_=bias_d)
        wqTb = [wAll[:, c * 256:(c + 1) * 256] for c in range(4)]
        wkTb = [wAll[:, 1024 + c * 256:1024 + (c + 1) * 256] for c in range(4)]
        wvTb = [wAll[:, 2048 + c * 256:2048 + (c + 1) * 256] for c in range(4)]
        woTb = [wAll[:, 3072 + p * 512:3072 + (p + 1) * 512] for p in range(2)]
        bqs = [biasT[:, p:p + 1] for p in range(2)]
        bks = [biasT[:, 2 + p:3 + p] for p in range(2)]

        # ------------- streaming input loads ------------------------------
        # mask chunks stream on the SWDGE queue (issued first so chunk 0
        # lands as early as possible); the HWDGE queue carries window 0,
        # then qT, then windows 1-7. Each kT window is one 512KB transfer
        # with 4KB contiguous lines (the four 128-row D-blocks side by
        # side in the free axis) so the DGE uses fat packets.
        Mb = persist.tile([128, NCHUNK * E], BF16, tag="Mb")
        kWins = [persist.tile([128, 2048], BF16, tag=f"kW{w}", name=f"kW{w}")
                 for w in range(8)]
        qAll = persist.tile([128, 4096], BF16, tag="qAll")
        qTb = [qAll[:, c * E:(c + 1) * E] for c in range(4)]
        nc.gpsimd.dma_start(out=qAll[:, 2 * E:], in_=qT_d[:, 2 * E:])
        for n in range(NCHUNK // 2):
            nc.gpsimd.dma_start(out=Mb[:, n * 2 * E:(n + 1) * 2 * E],
                                in_=mB_d[n])
        nc.sync.dma_start(out=qAll[:, 0:2 * E], in_=qT_d[:, 0:2 * E])
        for w in range(8):
            nc.sync.dma_start(out=kWins[w], in_=kTw_d[w])

        # ---------------- Q projection ----------------
        # QTs[l] (128, 1024) bf16: rows [64r, 64r+64) = head l's Q^T, rest 0
        # (l = 2p + r), so scores matmuls contract over the full 128
        # partitions against KTs[p]. Pair 1's Q/K projections are deferred
        # into the ACT-bound head-1/2 streams (only heads 2,3 need them).
        def proj_q(p):
            qp = ps.tile([128, E], F32, tag="st", bufs=2, name=f"qp{p}")
            for c in range(4):
                for e2 in range(2):
                    nc.tensor.matmul(
                        qp[:, e2 * 512:(e2 + 1) * 512],
                        wqTb[c][:, p * 128:(p + 1) * 128],
                        qTb[c][:, e2 * 512:(e2 + 1) * 512],
                        start=(c == 0), stop=(c == 3))
            for r in range(2):
                sl = slice(64 * r, 64 * r + 64)
                nc.vector.tensor_scalar_add(QTs[2 * p + r][sl, :], qp[sl, :],
                                            bqs[p][sl, :])

        proj_q(0)

        # ------------- K/V projections -----------------------------------
        KTs = [persist.tile([128, N], BF16, tag=f"KTs{p}", name=f"KTs{p}")
               for p in range(2)]
        pairN = [persist.tile([128, E], BF16, tag=f"pairN{p}", name=f"pairN{p}")
                 for p in range(2)]

        def proj_k(w, p):
            kp = ps.tile([128, 512], F32, tag="st", bufs=2, name=f"kp{p}_{w}")
            for c in range(4):
                nc.tensor.matmul(
                    kp, wkTb[c][:, p * 128:(p + 1) * 128],
                    kWins[w][:, c * 512:(c + 1) * 512],
                    start=(c == 0), stop=(c == 3))
            nc.vector.tensor_scalar_add(
                KTs[p][:, w * 512:(w + 1) * 512], kp, bks[p])

        def proj_v(n):
            w, j = divmod(n, 4)
            vp = ps.tile([128, 256], F32, tag="st", bufs=2, name=f"vp{n}")
            for c in range(4):
                blk = kWins[w][:, c * 512 + j * 128:c * 512 + j * 128 + 128]
                nc.tensor.matmul(vp, blk,
                                 wvTb[c], start=(c == 0), stop=(c == 3))
            dst = Vs4[:, n, :, 0:64]
            src = vp.rearrange("p (h c) -> p h c", h=4)
            nc.vector.tensor_copy(dst, src)

        # ------------- attention helpers ---------------------------------
        oTs = {}
        Ps = {}

        def score_part(l, n):
            # scores + exp + mask for (head l, node chunk n) -> P^T in Ps
            p = l // 2
            st = ps.tile([128, E], F32, tag="st", bufs=2, name=f"st{l}_{n}")
            kblk = KTs[p][:, n * 128:(n + 1) * 128]
            for e2 in range(2):
                sl = slice(e2 * 512, (e2 + 1) * 512)
                nc.tensor.matmul(st[:, sl], kblk, QTs[l][:, sl],
                                 start=True, stop=True)
            Praw = work.tile([128, E], BF16, tag="Praw", bufs=5,
                             name=f"Praw{l}_{n}")
            nc.scalar.activation(Praw, st, mybir.ActivationFunctionType.Exp,
                                 bias=0.0, scale=0.125)
            P = work.tile([128, E], BF16, tag="P", bufs=5, name=f"P{l}_{n}")
            nc.vector.tensor_mul(P, Praw, Mb[:, n * E:(n + 1) * E])
            Ps[(l, n)] = P

        def av_part(l, n):
            # attn @ V' for (head l, node chunk n), accumulating into oTs[l]
            P = Ps.pop((l, n))
            vblk = Vs4[:, n, l]
            for e2 in range(2):
                sl = slice(e2 * 512, (e2 + 1) * 512)
                nc.tensor.matmul(oTs[l][:, sl], vblk, P[:, sl],
                                 start=(n == 0), stop=(n == NCHUNK - 1))

        def head_seq(l):
            seq = []
            for n in range(NCHUNK):
                seq.append(lambda l=l, n=n: score_part(l, n))
                if n > 0:
                    seq.append(lambda l=l, n=n - 1: av_part(l, n))
            seq.append(lambda l=l: av_part(l, NCHUNK - 1))
            return seq

        # ------------- normalization (via DRAM bounces, as baseline) -----
        norm_state = {}

        def norm_stage1(l):
            # copy the exp-sum row out of PSUM, bounce to DRAM and back
            # reshaped (64, 16) so the reciprocal runs 64 lanes wide
            sums = work.tile([1, E], F32, tag="sums", bufs=2, name=f"sums{l}")
            nc.vector.tensor_copy(sums, oTs[l][64:65, :])
            sums_d = dpool.tile([1, E], F32, tag="sums_d", bufs=2,
                                name=f"sums_d{l}")
            nc.sync.dma_start(out=sums_d, in_=sums)
            sums64 = work.tile([64, 16], F32, tag="sums64", bufs=2,
                               name=f"sums64{l}")
            nc.sync.dma_start(
                out=sums64, in_=sums_d.rearrange("one (p k) -> (one p) k", p=64))
            norm_state[l] = sums64

        def norm_stage2(l):
            sums64 = norm_state.pop(l)
            recip64 = work.tile([64, 16], F32, tag="recip64", bufs=2,
                                name=f"recip64{l}")
            nc.vector.reciprocal(recip64, sums64)
            rec_d = dpool.tile([64, 16], F32, tag="rec_d", bufs=2,
                               name=f"rec_d{l}")
            nc.sync.dma_start(out=rec_d, in_=recip64)
            norm_state[l] = rec_d

        def norm_stage3(l):
            rec_row = norm_state[l].rearrange("p k -> (p k)").unsqueeze(0)
            recb = work.tile([64, E], F32, tag="recb", bufs=2, name=f"recb{l}")
            nc.sync.dma_start(out=recb, in_=rec_row.to_broadcast((64, E)))
            norm_state[l] = recb

        def norm_stage4(l):
            p, r = l // 2, l % 2
            recb = norm_state.pop(l)
            nc.vector.tensor_mul(pairN[p][64 * r:64 * r + 64, :],
                                 oTs[l][0:64, :], recb)

        NORM_STAGES = (norm_stage1, norm_stage2, norm_stage3, norm_stage4)

        def normalize_fast(l):
            # zero-DRAM normalization for the drain: broadcast the sums row
            # across 64 partitions with an f32r matmul into a free PSUM
            # slot, then 1/s = Exp(-Ln(s)) on the (idle) scalar engine
            p, r = l // 2, l % 2
            sums_sb = work.tile([1, E], mybir.dt.float32r, tag="sums",
                                bufs=2, name=f"sumsf{l}")
            nc.vector.tensor_copy(sums_sb, oTs[l][64:65, :])
            s_bc = ps.tile([64, E], F32, tag="st", bufs=2, name=f"sbc{l}")
            for e2 in range(2):
                sl = slice(e2 * 512, (e2 + 1) * 512)
                nc.tensor.matmul(s_bc[:, sl], ones_row, sums_sb[:, sl],
                                 start=True, stop=True)
            lnv = work.tile([64, E], F32, tag="recb", bufs=2, name=f"ln{l}")
            nc.scalar.activation(lnv, s_bc, mybir.ActivationFunctionType.Ln)
            recb = work.tile([64, E], F32, tag="recb", bufs=2,
                             name=f"recf{l}")
            nc.scalar.activation(recb, lnv, mybir.ActivationFunctionType.Exp,
                                 bias=0.0, scale=-1.0)
            nc.vector.tensor_mul(pairN[p][64 * r:64 * r + 64, :],
                                 oTs[l][0:64, :], recb)

        # ------------- merged pipeline -----------------------------------
        for l in (0, 1):
            oTs[l] = ps.tile([65, E], F32, tag="outT", bufs=2, name=f"oT{l}")

        # head 0 trails the K/V projection windows by one window
        proj_q(1)
        h0 = head_seq(0)
        h0i = 0
        for w in range(8):
            steps = [lambda w=w: proj_k(w, 0), lambda w=w: proj_k(w, 1)] + \
                    [lambda n=n: proj_v(n) for n in range(4 * w, 4 * w + 4)]
            for i, step in enumerate(steps):
                if w > 0 and i < 5 and h0i < len(h0):
                    h0[h0i]()
                    h0i += 1
                step()
        while h0i < len(h0):
            h0[h0i]()
            h0i += 1

        # heads 1-3; head l-1's normalization stages woven into head l's
        # stream so their latency hides
        for l in (1, 2, 3):
            if l >= 2:
                oTs[l] = ps.tile([65, E], F32, tag="outT", bufs=2,
                                 name=f"oT{l}")
            stages = {2: 0, 8: 1, 14: 2, 20: 3}
            for idx, item in enumerate(head_seq(l)):
                item()
                if idx in stages:
                    NORM_STAGES[stages[idx]](l - 1)

        # ---------------- output projection (partial) --------------------
        # chunk 0's pair-0 half pre-issues into the free outT slot, the fast
        # norm chain runs without DRAM hops, and warm matmuls keep the PE
        # clock up through the chain's latency
        fins = {}
        f0 = ps.tile([128, 512], F32, tag="outT", bufs=2, name="fin0")
        nc.tensor.matmul(f0, pairN[0][:, 0:128], woTb[0],
                         start=True, stop=False)
        fins[0] = f0
        normalize_fast(3)
        warm = ps.tile([128, 512], F32, tag="st", bufs=2, name="warm")
        for i in range(14):
            nc.tensor.matmul(warm, pairN[0][:, 0:128], woTb[0],
                             start=True, stop=True, skip_group_check=True)
        for e in range(ECHUNK):
            if e in fins:
                f = fins[e]
            else:
                f = ps.tile([128, 512], F32, tag=("st" if e % 2 else "outT"),
                            bufs=2, name=f"fin{e}")
                nc.tensor.matmul(f, pairN[0][:, e * 128:(e + 1) * 128],
                                 woTb[0], start=True, stop=False)
            nc.tensor.matmul(f, pairN[1][:, e * 128:(e + 1) * 128], woTb[1],
                             start=False, stop=True)
            fo = work.tile([128, 512], F32, tag="fo", bufs=4, name=f"fo{e}")
            nc.vector.tensor_copy(fo, f)
            nc.sync.dma_start(out=out_d[e * 128:(e + 1) * 128, :], in_=fo)

    nc.compile()
    return nc


def _get_nc():
    global _CACHED_NC
    if _CACHED_NC is None:
        _CACHED_NC = _build_nc()
    return _CACHED_NC


def _make_in_maps(queries, keys, incidence_matrix, Wq, bq, Wk, bk, Wv, bv, Wo, bo):
    """Host-side sharding + layout marshalling (transposes + bf16 casts)."""
    queries = np.asarray(queries, dtype=np.float32)
    keys = np.asarray(keys, dtype=np.float32)
    incidence = np.asarray(incidence_matrix, dtype=np.float32)
    Wq = np.asarray(Wq, dtype=np.float32)
    Wk = np.asarray(Wk, dtype=np.float32)
    Wv = np.asarray(Wv, dtype=np.float32)
    Wo = np.asarray(Wo, dtype=np.float32)
    bq = np.asarray(bq, dtype=np.float32)
    bk = np.asarray(bk, dtype=np.float32)

    per_batch = {}
    for b in range(BS):
        qT = np.ascontiguousarray(
            queries[b].T.reshape(4, 128, E).transpose(1, 0, 2).reshape(
                128, 4096))
        kT = np.ascontiguousarray(keys[b].T)
        kTw = np.ascontiguousarray(
            kT.reshape(4, 128, 8, 512).transpose(2, 1, 0, 3).reshape(
                8, 128, 2048))
        mB = np.ascontiguousarray(
            incidence[b].T.reshape(NCHUNK // 2, 2, 128, E).transpose(
                0, 2, 1, 3).reshape(NCHUNK // 2, 128, 2 * E))
        per_batch[b] = (qT.astype(BF16NP), kTw.astype(BF16NP),
                        mB.astype(BF16NP))

    in_maps = []
    for core in range(8):
        b, g = core // 2, core % 2
        sl = slice(g * 256, (g + 1) * 256)
        qT8, kTw8, mB8 = per_batch[b]
        wAll = np.zeros((128, 4096), np.float32)
        for c in range(4):
            wAll[:, c * 256:(c + 1) * 256] = Wq[sl, :].T[c * 128:(c + 1) * 128]
            wAll[:, 1024 + c * 256:1024 + (c + 1) * 256] = \
                Wk[sl, :].T[c * 128:(c + 1) * 128]
            wAll[:, 2048 + c * 256:2048 + (c + 1) * 256] = \
                Wv[sl, :].T[c * 128:(c + 1) * 128]
        woT = Wo[:, sl].T
        wAll[:, 3072:3584] = woT[0:128]
        wAll[:, 3584:4096] = woT[128:256]
        wAll = wAll.astype(BF16NP)
        bias4 = np.stack([bq[sl][0:128], bq[sl][128:256],
                          bk[sl][0:128], bk[sl][128:256]], axis=1).astype(
            np.float32).copy()
        in_maps.append({
            "qT": qT8,
            "kTw": kTw8,
            "mB": mB8,
            "wAll": wAll,
            "bias4": bias4,
        })
    return in_maps


def kernel(**inputs):
    global LAST_EXEC_TIME_NS
    nc = _get_nc()
    in_maps = _make_in_maps(**inputs)
    trace = bool(os.environ.get("BASS_TRACE"))
    if trace:
        _install_ntff_hook()
    res = run_bass_kernel_spmd(nc, in_maps, core_ids=list(range(8)), trace=trace)
    LAST_EXEC_TIME_NS = res.exec_time_ns
    out = np.zeros((BS, E, D), np.float32)
    # attention rows sum to 1, so the V bias contributes bv @ Wo.T exactly;
    # add it (and bo) once here instead of on the device
    bias = (np.asarray(inputs["bo"], np.float32)
            + np.asarray(inputs["bv"], np.float32)
            @ np.asarray(inputs["Wo"], np.float32).T)
    for b in range(BS):
        out[b] = res.results[2 * b]["out"] + res.results[2 * b + 1]["out"] + bias
    return out


def _install_ntff_hook():
    """Recreate the missing antenv.axon_hooks glue so trace=True captures NTFF."""
    import types
    if "antenv.axon_hooks" in sys.modules:
        return
    try:
        from trn_agent_boot.trn_boot import _ntff_profile_via_ctypes
        hook = _ntff_profile_via_ctypes("/opt/axon/libaxon_pjrt.so")
        m = types.ModuleType("antenv.axon_hooks")
        m.get_axon_ntff_profile_hook = lambda: hook
        m.set_axon_ntff_profile_hook = lambda h: None
        sys.modules["antenv.axon_hooks"] = m
    except Exception:
        pass
